# revision 1
# baseline (speedup 1.0000x reference)
"""Trainium2 Bass kernel for nn_ClusterLoss_Regr (topk_masking).

Computes  mean_b(128 - max_p((128 - d[b,p]) * [|proto[p] - label[b]| <= 0.5]))
for d: [8192, 4096] f32, labels: [8192] f32, proto: [4096] f32 -> scalar f32.

Sharding: data-parallel over the batch axis across 8 NeuronCores (1024 rows
per core); proto_classes replicated; final mean on host.

Device strategy (memory-bound):
  - d is staged to HBM as bf16 (halves HBM traffic; rel tolerance 2e-2 vs
    bf16's 2^-9 rounding).  Since f32/bf16 rounding is monotone,
    max_p f32(128-d_p) = f32(128 - min_p d_p), so the device computes the
    masked row MIN of d directly and the host reconstructs
    f32(128 - f32(128 - dmin)) bit-exactly mirroring the reference chain.
  - One fused custom-DVE op per row-tile:
        v[p,k]   = select(0.5 >= |proto[k] - label[p]|, d[p,k], BIG)
        accum[p] = min_k v[p,k]
    The op is registered with a hand-written 2X_1PORT uop program (packed
    bf16 pairs, SRC_0_HI/SRC_1_HI crossbar inputs, dual lo/hi ALU chains in
    8 stages) so the DVE runs at 2 elem/lane/cycle; the label rides latched
    swap-flops at the two ABSOLUTE_DIFF stages.
  - proto is staged pre-replicated [128, 4096] bf16 by the host (1 MB extra
    HBM read, fully overlapped) replacing the baseline's 14.5 us GPSIMD
    partition_broadcast prologue.
  - raw Bass with manual semaphores: head DMAs ride the scalar HWDGE ring;
    d-tile DMAs stream back-to-back on the sync ring; DVE op t is gated only
    on d-tile t; the last row-tile is processed as 4 quarter-width ops to
    minimise the tail.
Host: gather [8192] row minima, loss = mean(128 - (128 - dmin)) accumulated
in f64, cast to f32.
"""

import numpy as np

B, P = 8192, 4096
NCORES = 8
BSH = B // NCORES  # 1024 rows per core
RT = BSH // 128    # 8 row-tiles of 128 rows
MAX_DIST = np.float32(128.0)
BIG = 2.0          # "unmasked" fill; any value > max(d)=1.0 works
USE_2X = True

_cache: dict = {}


def _ensure_path():
    try:
        import concourse.bass  # noqa: F401
    except ImportError:
        import sys

        for p in ("/opt/trn_rl_repo",):
            if p not in sys.path:
                sys.path.insert(0, p)


def _build_2x_uops():
    """Hand-written 2X_1PORT program: 2 states (seed, steady) rate-matched
    to lower()'s 1x program, shaped after the stock tensor_mask 2x program
    (slot 105 of the gen3 firmware table): SRC_0 rides input 0 (read at b0
    as PREV_ALU_OUT, captured into lane 0), both write halves enabled.

    Lanes: L0=SRC_1 (b0 only; then captures Src0, then v_lo), L1=CONST_0
    (label), L2=CONST_1 (0.5), L3=CONST_2 (BIG), L4=SRC_0_HI (then v_hi
    from b6), L5=SRC_1_HI.

      b0: ad_lo = |Src1 - label|   [d0 <- Src0]
      b1: c_lo  = 0.5 >= ad_lo
      b2: v_lo  = sel(c_lo, Src0, BIG)
      b3: ad_hi = |Src1_HI - label|  [d0 <- v_lo]
      b4: c_hi  = 0.5 >= ad_hi
      b5: v_hi  = sel(c_hi, Src0_HI, BIG)
      b6: w     = min(v_lo, v_hi)
      b7: acc   = min(acc, w)  -> written to both output halves; the last
                  written word of the stream is the masked row min.  (The
                  persistent-accumulator readback is dead in 2X mode, so the
                  running min is streamed through the write port instead.)
    """
    from concourse.dve_uop import (
        AluInp,
        AluOp,
        DelayInp,
        InpSel,
        OutPath,
        OutSel,
        Trigger,
        UopConfig,
        UopDpConfig,
    )

    ENABLE = 1
    P_AD = AluInp.PREV_ALU_OUT
    CUR = AluInp.CURR_ALU_OUT
    D = [
        AluInp.PREV_DELAY_0,
        AluInp.PREV_DELAY_1,
        AluInp.PREV_DELAY_2,
        AluInp.PREV_DELAY_3,
        AluInp.PREV_DELAY_4,
        AluInp.PREV_DELAY_5,
    ]
    SRC_DONE = (Trigger.SRC_TENSOR_DONE, Trigger.NONE, Trigger.NONE)
    COUNT_ONCE = (Trigger.COUNT, Trigger.NONE, Trigger.NONE)

    def wire_inputs(u):
        u.enable_input(InpSel.SRC_0, 0)      # input 0 -> b0's PREV_ALU_OUT
        u.enable_input(InpSel.SRC_1, 1)      # lane 0
        u.enable_input(InpSel.CONST_0, 2)    # lane 1: label
        u.enable_input(InpSel.CONST_1, 3)    # lane 2: 0.5
        u.enable_input(InpSel.CONST_2, 4)    # lane 3: BIG
        u.enable_input(InpSel.SRC_0_HI, 5)   # lane 4
        u.enable_input(InpSel.SRC_1_HI, 6)   # lane 5

    def steady_blocks():
        dp = [UopDpConfig() for _ in range(8)]
        for i in range(8):
            dp[i].pass_through_delay(1, 2, 3, 5)
            if i not in (0, 3):
                dp[i].pass_through_delay(0)
            if i != 6:
                dp[i].pass_through_delay(4)
        dp[0].enable_alu(AluOp.ABSOLUTE_DIFF, D[0], D[1])
        dp[0].enable_delay_from_src(DelayInp.PREV_ALU_OUT, 0)  # Src0
        dp[1].enable_alu(AluOp.IS_GE, D[2], P_AD)
        dp[2].enable_alu(AluOp.SELECT, D[3], D[0])  # falsy->BIG, truthy->Src0
        dp[3].enable_alu(AluOp.ABSOLUTE_DIFF, D[5], D[1])
        dp[3].enable_delay_from_src(DelayInp.PREV_ALU_OUT, 0)  # v_lo
        dp[4].enable_alu(AluOp.IS_GE, D[2], P_AD)
        dp[5].enable_alu(AluOp.SELECT, D[3], D[4])
        dp[6].enable_alu(AluOp.MIN, D[0], P_AD)
        dp[7].enable_alu(AluOp.MIN, CUR, P_AD)
        return dp

    # --- state 0: seed — scan state (b7's out-flop) <- BIG (CONST_2, lane 3).
    sd = UopConfig(trigger=COUNT_ONCE, repeat_count=1, next_uop=(1, 0, 0))
    wire_inputs(sd)
    dp = steady_blocks()
    dp[7] = UopDpConfig()
    dp[7].pass_through_delay(0, 1, 2, 3, 4, 5)
    dp[7].enable_alu(AluOp.BYPASS, D[3], D[3])
    sd.datapath_config = dp

    # --- state 1: steady.  Both write halves carry the running min.
    st = UopConfig(
        trigger=SRC_DONE,
        require_inp0=ENABLE,
        require_inp1=ENABLE,
    )
    wire_inputs(st)
    st.datapath_config = steady_blocks()
    st.enable_output(OutSel.ALU_OUT, OutPath.WR0_LO)
    st.enable_output(OutSel.ALU_OUT, OutPath.WR0_HI)
    return [sd, st]


def _register_dve_op():
    """Register the fused |proto-label|-mask + min-reduce op, with a
    hand-authored 2X_1PORT perf-mode program. Idempotent."""
    from concourse import dve_ops
    from concourse.dve_spec import (
        C0,
        C1,
        C2,
        AluOp,
        Bin,
        Spec,
        Src0,
        Src1,
        lower,
        scan,
        select,
    )
    from concourse.dve_uop import DveOpSpec

    name = "CLUSTER_MASKMIN_ANT"
    for op in dve_ops.OPS:
        if op.name == name:
            return op

    def _ref(in0, in1, s0, s1, imm2):
        mask = np.abs(in1.astype(np.float32) - np.asarray(s0, np.float32)) <= (
            np.float32(s1)
        )
        o = np.where(mask, in0.astype(np.float32), np.float32(imm2)).astype(
            np.float32
        )
        return np.minimum.accumulate(o, axis=-1)

    # body: running min of select(0.5 >= |Src1 - label|, Src0, BIG); the
    # last element of the output stream is the masked row min.
    spec = Spec(
        body=scan(
            AluOp.MIN,
            select(C1 >= Bin(AluOp.ABSOLUTE_DIFF, Src1, C0), Src0, C2),
            init=C2,
        ),
        reference=_ref,
    )

    class _DveOp2x(dve_ops.DveOp):
        def compile(self, ver):
            key = (self.name, ver)
            if (r := dve_ops._COMPILE_CACHE.get(key)) is not None:
                return r
            uops = lower(self.spec, ver=ver)
            uops_2x = None
            if USE_2X and ver == "v3":
                uops_2x = _build_2x_uops()
                assert len(uops_2x) == len(uops), (len(uops_2x), len(uops))
            result = DveOpSpec(
                name=self.name,
                opcode=dve_ops.get_dve_sub_opcode(self.name),
                uops=uops,
                uops_2x=uops_2x,
                perf_max=1 if uops_2x is not None else 0,
                rd1_en=True,
            )
            dve_ops._COMPILE_CACHE[key] = result
            return result

    shas: dict = {}
    op = _DveOp2x(name, spec, subdim=False, uops_sha=shas)
    dve_ops.OPS.append(op)
    row = dve_ops._CUSTOM_DVE_ROW_BASE + len(dve_ops.OPS) - 1
    dve_ops._SUB_OPCODE_FOR_NAME[name] = row
    dve_ops.CUSTOM_DVE_SPECS[name] = spec
    for ver in ("v3", "v4"):
        shas[ver] = op.compile(ver).sha(ver) if ver == "v3" else ""
    return op


def _get_bass():
    if "nc" in _cache:
        return _cache["nc"]
    _ensure_path()
    import concourse.bacc as bacc
    import concourse.mybir as mybir

    op = _register_dve_op()
    f32 = mybir.dt.float32
    bf16 = mybir.dt.bfloat16
    nc = bacc.Bacc(
        "TRN2", target_bir_lowering=False, debug=False, num_devices=NCORES
    )
    d_ap = nc.dram_tensor("d", [128, RT * P], bf16, kind="ExternalInput").ap()
    lab_ap = nc.dram_tensor("labels_col", [128, RT], f32, kind="ExternalInput").ap()
    pb_ap = nc.dram_tensor("proto_row", [1, P], bf16, kind="ExternalInput").ap()
    # The LAST row-tile is processed as NSPLIT quarter-width ops so the
    # final DVE op rides only a quarter tile behind the last DMA byte.
    NSPLIT = 4
    ND = RT - 1 + NSPLIT      # number of d DMAs == number of DVE ops
    NV = ND
    # rowmin[:, 2i:2i+2] <- the last written word of op i's scan stream;
    # column 2i+1 is the final running-min in both 1x and 2x modes.
    out_ap = nc.dram_tensor("rowmin", [128, 2 * ND], bf16, kind="ExternalOutput").ap()

    proto_tile = nc.alloc_sbuf_tensor("proto_tile", [128, P], bf16).ap()
    prow = nc.alloc_sbuf_tensor("prow", [1, P], bf16).ap()
    ones = nc.alloc_sbuf_tensor("ones", [1, 128], bf16).ap()
    psum = [
        nc.alloc_psum_tensor(f"pbc{b}", [128, 512], mybir.dt.float32).ap()
        for b in range(P // 512)
    ]
    labels_tile = nc.alloc_sbuf_tensor("labels_tile", [128, RT], f32).ap()
    # per-tile scan output (ops into the same tile use disjoint col ranges)
    scr = [nc.alloc_sbuf_tensor(f"scr{t}", [128, P], bf16).ap() for t in range(RT)]
    dbig = nc.alloc_sbuf_tensor("dbig", [128, RT * P], bf16).ap()

    H = P // NSPLIT           # split width of the last tile

    # (tile, col_lo, width) in stream order; col offsets are into dbig
    work = [(t, 0, P) for t in range(RT - 1)]
    for s in range(NSPLIT):
        work.append((RT - 1, s * H, H))
    # one d DMA per work item (1 MB per full tile; mid-stream rate is
    # identical to fused 2 MB transfers, and per-item gating is simplest)
    dma_plan = [(t * P + lo, w, i) for i, (t, lo, w) in enumerate(work)]
    # op i is gated on the dma covering it
    dma_of_op = {}
    for di, (lo, w, first_op) in enumerate(dma_plan):
        for oi in range(len(work)):
            t, olo, ow = work[oi]
            a = t * P + olo
            if lo <= a < lo + w:
                dma_of_op[oi] = max(dma_of_op.get(oi, 0), di)

    # One semaphore per DMA (a shared sem with cumulative thresholds can
    # fire early when the 16 SDMA engines skew; a dedicated sem == 16 is
    # exact).
    d_sems = [nc.alloc_semaphore(f"d_sem{i}") for i in range(len(dma_plan))]
    pb_sem = nc.alloc_semaphore("pb_sem")
    prow_sem = nc.alloc_semaphore("prow_sem")
    ones_sem = nc.alloc_semaphore("ones_sem")
    mm_sem = nc.alloc_semaphore("mm_sem")
    lab_sem = nc.alloc_semaphore("lab_sem")
    out_sem = nc.alloc_semaphore("out_sem")
    dve_sem = nc.alloc_semaphore("dve_sem")

    with nc.Block() as block:

        @block.sync
        def _(sync):
            # All DMAs ride the single sync HWDGE ring: a separate
            # scalar-ring DMA is starved ~1:12 behind the d-stream by the
            # per-queue-row round-robin (measured: 17 us for 1 MB).  proto
            # goes up as one 8 KB row; the broadcast to 128 partitions runs
            # on the idle PE (ones[128,1] outer product -> PSUM, 512-col
            # slices) + scalar engine (PSUM -> SBUF bf16 copy), keeping the
            # 1 MB replica off both HBM and the DVE.
            sync.dma_start(prow[:], pb_ap[:]).then_inc(prow_sem, 16)
            sync.dma_start(labels_tile[:], lab_ap[:]).then_inc(lab_sem, 16)
            for i, (lo, w, _fo) in enumerate(dma_plan):
                sync.dma_start(
                    dbig[:, lo : lo + w], d_ap[:, lo : lo + w]
                ).then_inc(d_sems[i], 16)
            # Gather each op's final scan word as soon as that op retires;
            # only the last one's completion latency lands in the tail.
            for i, (t, lo, w) in enumerate(work):
                sync.wait_ge(dve_sem, i + 1)
                sync.dma_start(
                    out_ap[:, 2 * i : 2 * i + 2],
                    scr[t][:, lo + w - 2 : lo + w],
                ).then_inc(out_sem, 16)
            sync.wait_ge(out_sem, 16 * ND)
            # Reset all kernel semaphores so re-executing the loaded NEFF
            # behaves identically to the first run.
            all_sems = sorted(
                s.num
                for s in [
                    *d_sems, pb_sem, prow_sem, ones_sem, mm_sem, lab_sem,
                    out_sem, dve_sem,
                ]
            )
            lo = prev = all_sems[0]
            for n in all_sems[1:] + [None]:
                if n is not None and n == prev + 1:
                    prev = n
                    continue
                sync.sem_clear(range(lo, prev + 1))
                if n is not None:
                    lo = prev = n

        @block.gpsimd
        def _(gpsimd):
            gpsimd.memset(ones[:], 1.0)
            gpsimd.sem_inc(ones_sem, 1)

        @block.tensor
        def _(tensor):
            tensor.wait_ge(prow_sem, 16)
            tensor.wait_ge(ones_sem, 1)
            for b in range(P // 512):
                tensor.matmul(
                    psum[b][:, :],
                    ones[:, :],
                    prow[:, 512 * b : 512 * (b + 1)],
                ).then_inc(mm_sem, 1)

        @block.scalar
        def _(scalar):
            for b in range(P // 512):
                scalar.wait_ge(mm_sem, b + 1)
                scalar.copy(
                    proto_tile[:, 512 * b : 512 * (b + 1)], psum[b][:, :]
                ).then_inc(pb_sem, 1)

        @block.vector
        def _(vector):
            vector.wait_ge(pb_sem, P // 512)
            vector.wait_ge(lab_sem, 16)
            for i, (t, lo, w) in enumerate(work):
                vector.wait_ge(d_sems[dma_of_op[i]], 16)
                inst = nc.vector._custom_dve(
                    op,
                    out=scr[t][:, lo : lo + w],
                    in0=dbig[:, t * P + lo : t * P + lo + w],
                    in1=proto_tile[:, lo : lo + w],
                    s0=labels_tile[:, t : t + 1],
                    s1=0.5,
                    imm2=float(BIG),
                )
                if USE_2X:
                    inst.ins.perf_max = 1
                inst.then_inc(dve_sem, 1)

    nc.compile()
    _cache["nc"] = nc
    return nc


def _prep_inputs(min_distances, labels, proto_classes):
    import ml_dtypes

    bf16 = ml_dtypes.bfloat16
    d = np.asarray(min_distances, dtype=np.float32).astype(bf16)
    proto = np.asarray(proto_classes, dtype=np.float32).astype(bf16)
    proto_row = np.ascontiguousarray(proto[None, :])
    labf = np.asarray(labels, dtype=np.float32)
    in_maps = []
    for c in range(NCORES):
        dsh = np.ascontiguousarray(
            d[c * BSH : (c + 1) * BSH]
            .reshape(RT, 128, P)
            .transpose(1, 0, 2)
            .reshape(128, RT * P)
        )
        lsh = np.ascontiguousarray(
            labf[c * BSH : (c + 1) * BSH].reshape(RT, 128).T
        )
        in_maps.append({"d": dsh, "labels_col": lsh, "proto_row": proto_row})
    return in_maps


def _run_device(min_distances, labels, proto_classes, trace=False):
    nc = _get_bass()
    from concourse.bass_utils import run_bass_kernel_spmd

    in_maps = _prep_inputs(min_distances, labels, proto_classes)
    return run_bass_kernel_spmd(
        nc, in_maps, core_ids=list(range(NCORES)), trace=trace
    )


def kernel(min_distances, labels, proto_classes):
    res = _run_device(min_distances, labels, proto_classes).results
    # rowmin[:, 2i+1] = final scan value of op i.  Ops 0..RT-2 are tiles
    # 0..RT-2; ops RT-1.. are quarters of tile RT-1 (combine by min).
    # Row = 1024*c + 128*t + p.  bf16 is exact here: a min of bf16 values.
    stats = np.stack(
        [
            np.asarray(res[c]["rowmin"])[:, 1::2].astype(np.float32)
            for c in range(NCORES)
        ]
    )
    t_last = stats[:, :, RT - 1 :].min(axis=2)
    rowmin = np.concatenate([stats[:, :, : RT - 1], t_last[:, :, None]], axis=2)
    rowmin = rowmin.transpose(0, 2, 1).reshape(B).astype(np.float32)
    # mirror the reference's f32 rounding chain exactly:
    # loss_row = f32(128 - f32(128 - dmin))
    inv = (MAX_DIST - rowmin).astype(np.float32)
    loss_rows = (MAX_DIST - inv).astype(np.float32)
    return np.array(loss_rows.mean(dtype=np.float64), dtype=np.float32)



# revision 3
# speedup vs baseline: 1.1075x; 1.1075x over previous
"""Trainium2 Bass kernel for nn_ClusterLoss_Regr (topk_masking) — v2.

Computes  mean_b(128 - max_p((128 - d[b,p]) * [|proto[p] - label[b]| <= 0.5]))
for d: [8192, 4096] f32, labels: [8192] f32, proto: [4096] f32 -> scalar f32.

v2 design ("sorted staircase"):
  - Host sorts columns by proto value and rows by label.  Each row's mask is
    then one contiguous column range [lo_r, hi_r).  Rows are grouped into 64
    groups of 128 consecutive sorted rows; groups are rank-bucketed by their
    union mask width so the 8 groups of tile-index t have near-equal widths
    (uniform compile-time shapes across the 8 SPMD cores).  Columns outside
    a tile's union range are never staged or read (~21% HBM traffic cut).
  - Per 128-row tile the union range splits into an all-masked INTERIOR
    (every row of the tile wants these columns) and two narrow boundary
    BANDS where the mask varies by row.
  - INTERIOR: a custom single-source running-min DVE op with hand-written
    2X_1PORT / 2X_2PORT / 4X_2PORT uop programs -> 4 elem/lane/cycle.
    No mask needed, no proto data on device at all.
  - BANDS: the masked-min op compares an f16 index ramp against per-row
    range constants (|j - c0| <= 512 encodes j >= a_r or j < b_r), 2X mode.
    Mask bounds are computed EXACTLY on host (f32 predicate refinement).
  - All DVE ops write into one shared scratch buffer with descending end
    offsets (step 4) so every op's final running-min lands in a contiguous
    window -> ONE tail gather DMA instead of one per op.
  - d staged as bf16 (measured: plain bf16 HWDGE sustains ~395 GB/s/core;
    fp8 cast-DMA is write-side-bound and no faster).
Host: decode rowmins, map through the row permutation, mean in f64.
"""

import numpy as np

B, P = 8192, 4096
NCORES = 8
BSH = B // NCORES      # 1024 rows per core
RT = BSH // 128        # 8 row-tiles of 128 rows
NG = B // 128          # 64 row groups
ALIGN = 8
MAX_DIST = np.float32(128.0)
BIG = 2.0              # > max(d)=1.0
HALF_W = 512.0         # band compare halfwidth (band widths < 512)

_cache: dict = {}


def _ensure_path():
    try:
        import concourse.bass  # noqa: F401
    except ImportError:
        import sys

        for p in ("/opt/trn_rl_repo",):
            if p not in sys.path:
                sys.path.insert(0, p)


# --------------------------------------------------------------- DVE ops
def _build_maskmin_2x_uops():
    """2X_1PORT program for the masked-min op (see spec in _register_ops):
    v = select(C1 >= |Src1 - C0|, Src0, C2); acc = min(acc, v); acc streamed
    to both write halves.  Identical to the proven v1 program."""
    from concourse.dve_uop import (
        AluInp,
        AluOp,
        DelayInp,
        InpSel,
        OutPath,
        OutSel,
        Trigger,
        UopConfig,
        UopDpConfig,
    )

    ENABLE = 1
    P_AD = AluInp.PREV_ALU_OUT
    CUR = AluInp.CURR_ALU_OUT
    D = [
        AluInp.PREV_DELAY_0,
        AluInp.PREV_DELAY_1,
        AluInp.PREV_DELAY_2,
        AluInp.PREV_DELAY_3,
        AluInp.PREV_DELAY_4,
        AluInp.PREV_DELAY_5,
    ]
    SRC_DONE = (Trigger.SRC_TENSOR_DONE, Trigger.NONE, Trigger.NONE)
    COUNT_ONCE = (Trigger.COUNT, Trigger.NONE, Trigger.NONE)

    def wire_inputs(u):
        u.enable_input(InpSel.SRC_0, 0)      # input 0 -> b0's PREV_ALU_OUT
        u.enable_input(InpSel.SRC_1, 1)      # lane 0
        u.enable_input(InpSel.CONST_0, 2)    # lane 1: center
        u.enable_input(InpSel.CONST_1, 3)    # lane 2: halfwidth
        u.enable_input(InpSel.CONST_2, 4)    # lane 3: BIG
        u.enable_input(InpSel.SRC_0_HI, 5)   # lane 4
        u.enable_input(InpSel.SRC_1_HI, 6)   # lane 5

    def steady_blocks():
        dp = [UopDpConfig() for _ in range(8)]
        for i in range(8):
            dp[i].pass_through_delay(1, 2, 3, 5)
            if i not in (0, 3):
                dp[i].pass_through_delay(0)
            if i != 6:
                dp[i].pass_through_delay(4)
        dp[0].enable_alu(AluOp.ABSOLUTE_DIFF, D[0], D[1])
        dp[0].enable_delay_from_src(DelayInp.PREV_ALU_OUT, 0)  # Src0
        dp[1].enable_alu(AluOp.IS_GE, D[2], P_AD)
        dp[2].enable_alu(AluOp.SELECT, D[3], D[0])
        dp[3].enable_alu(AluOp.ABSOLUTE_DIFF, D[5], D[1])
        dp[3].enable_delay_from_src(DelayInp.PREV_ALU_OUT, 0)  # v_lo
        dp[4].enable_alu(AluOp.IS_GE, D[2], P_AD)
        dp[5].enable_alu(AluOp.SELECT, D[3], D[4])
        dp[6].enable_alu(AluOp.MIN, D[0], P_AD)
        dp[7].enable_alu(AluOp.MIN, CUR, P_AD)
        return dp

    sd = UopConfig(trigger=COUNT_ONCE, repeat_count=1, next_uop=(1, 0, 0))
    wire_inputs(sd)
    dp = steady_blocks()
    dp[7] = UopDpConfig()
    dp[7].pass_through_delay(0, 1, 2, 3, 4, 5)
    dp[7].enable_alu(AluOp.BYPASS, D[3], D[3])
    sd.datapath_config = dp

    st = UopConfig(trigger=SRC_DONE, require_inp0=ENABLE, require_inp1=ENABLE)
    wire_inputs(st)
    st.datapath_config = steady_blocks()
    st.enable_output(OutSel.ALU_OUT, OutPath.WR0_LO)
    st.enable_output(OutSel.ALU_OUT, OutPath.WR0_HI)
    return [sd, st]


def _build_scan_uops(nports):
    """Pure running-min programs.  nports=1 -> 2X_1PORT, 2 -> 2X_2PORT,
    4 -> 4X_2PORT.  Lanes: 0=SRC_0, 1=SRC_0_HI, 2=SRC_1, 3=SRC_1_HI,
    4=CONST_2 (seed).  state0 seeds blk7's out-flop; state1:
    acc = min(acc, min(elems)), streamed to the write ports."""
    from concourse.dve_uop import (
        AluInp,
        AluOp,
        DelayInp,
        InpSel,
        OutPath,
        OutSel,
        Trigger,
        UopConfig,
        UopDpConfig,
    )

    P_AD = AluInp.PREV_ALU_OUT
    CUR = AluInp.CURR_ALU_OUT
    D = [
        AluInp.PREV_DELAY_0,
        AluInp.PREV_DELAY_1,
        AluInp.PREV_DELAY_2,
        AluInp.PREV_DELAY_3,
        AluInp.PREV_DELAY_4,
        AluInp.PREV_DELAY_5,
    ]
    COUNT_ONCE = (Trigger.COUNT, Trigger.NONE, Trigger.NONE)
    SRC_DONE = (Trigger.SRC_TENSOR_DONE, Trigger.NONE, Trigger.NONE)

    def wire_inputs(u):
        u.enable_input(InpSel.SRC_0, 0)
        u.enable_input(InpSel.SRC_0_HI, 1)
        if nports >= 2:
            u.enable_input(InpSel.SRC_1, 2)
        if nports == 4:
            u.enable_input(InpSel.SRC_1_HI, 3)
        u.enable_input(InpSel.CONST_2, 4)

    def passthroughs():
        dp = [UopDpConfig() for _ in range(8)]
        for i in range(8):
            dp[i].pass_through_delay(0, 1, 2, 3, 4, 5)
        return dp

    def steady():
        dp = [UopDpConfig() for _ in range(8)]
        if nports == 4:
            dp[0].pass_through_delay(1, 2, 3, 4, 5)
            dp[0].enable_alu(AluOp.MIN, P_AD, D[0])        # m0 = min(A, B)
            dp[1].pass_through_delay(1, 2, 3, 4, 5)
            dp[1].enable_alu(AluOp.MIN, D[1], D[2])        # m1 = min(C, D)
            dp[1].enable_delay_from_src(DelayInp.PREV_ALU_OUT, 0)  # d0 <- m0
            dp[2].pass_through_delay(1, 2, 3, 4, 5)
            dp[2].enable_alu(AluOp.MIN, P_AD, D[0])        # m = min(m1, m0)
            first_bypass = 3
        elif nports == 2:
            dp[0].pass_through_delay(0, 1, 3, 4, 5)
            dp[0].enable_alu(AluOp.MIN, P_AD, D[1])        # min(A, C)
            first_bypass = 1
        else:
            dp[0].pass_through_delay(1, 2, 3, 4, 5)
            dp[0].enable_alu(AluOp.MIN, P_AD, D[0])        # min(A, B)
            first_bypass = 1
        for i in range(first_bypass, 7):
            dp[i].pass_through_delay(0, 1, 2, 3, 4, 5)
            dp[i].enable_alu(AluOp.BYPASS, P_AD, P_AD)
        dp[7].pass_through_delay(0, 1, 2, 3, 4, 5)
        dp[7].enable_alu(AluOp.MIN, CUR, P_AD)             # acc
        return dp

    sd = UopConfig(trigger=COUNT_ONCE, repeat_count=1, next_uop=(1, 0, 0))
    wire_inputs(sd)
    dp = passthroughs()
    dp[7] = UopDpConfig()
    dp[7].pass_through_delay(0, 1, 2, 4, 5)
    dp[7].enable_alu(AluOp.BYPASS, D[3], D[3])             # seed <- BIG
    sd.datapath_config = dp

    st = UopConfig(
        trigger=SRC_DONE,
        require_inp0=1,
        require_inp1=1 if nports >= 2 else 0,
    )
    wire_inputs(st)
    st.datapath_config = steady()
    st.enable_output(OutSel.ALU_OUT, OutPath.WR0_LO)
    st.enable_output(OutSel.ALU_OUT, OutPath.WR0_HI)
    if nports >= 2:
        st.enable_output(OutSel.ALU_OUT, OutPath.WR1_LO)
        st.enable_output(OutSel.ALU_OUT, OutPath.WR1_HI)
    return [sd, st]


def _register_ops():
    """Register the masked-min (2X) and pure-min-scan (4X) ops. Idempotent."""
    from concourse import dve_ops
    from concourse.dve_spec import (
        C0,
        C1,
        C2,
        AluOp,
        Bin,
        Spec,
        Src0,
        Src1,
        lower,
        scan,
        select,
    )
    from concourse.dve_uop import DveOpSpec

    def _make(name, spec, build_variants, perf_max, rd1_en):
        for op in dve_ops.OPS:
            if op.name == name:
                return op

        class _Op(dve_ops.DveOp):
            def compile(self, ver):
                key = (self.name, ver)
                if (r := dve_ops._COMPILE_CACHE.get(key)) is not None:
                    return r
                uops = lower(self.spec, ver=ver)
                variants = {}
                if ver == "v3":
                    variants = build_variants()
                    for v in variants.values():
                        assert len(v) == len(uops), (name, len(v), len(uops))
                result = DveOpSpec(
                    name=self.name,
                    opcode=dve_ops.get_dve_sub_opcode(self.name),
                    uops=uops,
                    uops_2x=variants.get("2x"),
                    uops_2x_2p=variants.get("2x_2p"),
                    uops_4x=variants.get("4x"),
                    perf_max=perf_max if variants else 0,
                    rd1_en=rd1_en,
                )
                dve_ops._COMPILE_CACHE[key] = result
                return result

        shas: dict = {}
        op = _Op(name, spec, subdim=False, uops_sha=shas)
        dve_ops.OPS.append(op)
        row = dve_ops._CUSTOM_DVE_ROW_BASE + len(dve_ops.OPS) - 1
        dve_ops._SUB_OPCODE_FOR_NAME[name] = row
        dve_ops.CUSTOM_DVE_SPECS[name] = spec
        for ver in ("v3", "v4"):
            shas[ver] = op.compile(ver).sha(ver) if ver == "v3" else ""
        return op

    def _mm_ref(in0, in1, s0, s1, imm2):
        mask = np.abs(in1.astype(np.float32) - np.asarray(s0, np.float32)) <= (
            np.float32(s1)
        )
        o = np.where(mask, in0.astype(np.float32), np.float32(imm2)).astype(
            np.float32
        )
        return np.minimum.accumulate(o, axis=-1)

    mm_spec = Spec(
        body=scan(
            AluOp.MIN,
            select(C1 >= Bin(AluOp.ABSOLUTE_DIFF, Src1, C0), Src0, C2),
            init=C2,
        ),
        reference=_mm_ref,
    )
    mm = _make(
        "CLUSTER_MASKMIN_ANT",
        mm_spec,
        lambda: {"2x": _build_maskmin_2x_uops()},
        perf_max=1,
        rd1_en=True,
    )

    def _sc_ref(in0, in1, s0, s1, imm2):
        o = np.minimum.accumulate(in0.astype(np.float32), axis=-1)
        return np.minimum(o, np.float32(imm2))

    sc_spec = Spec(body=scan(AluOp.MIN, Src0, init=C2), reference=_sc_ref)
    sc = _make(
        "PUREMIN_SCAN_ANT",
        sc_spec,
        lambda: {
            "2x": _build_scan_uops(1),
            "2x_2p": _build_scan_uops(2),
            "4x": _build_scan_uops(4),
        },
        perf_max=3,
        rd1_en=False,
    )
    return mm, sc


# --------------------------------------------------------------- host plan
def _exact_bounds(psort, lab):
    """Exact contiguous mask range per row: first/last sorted-proto index j
    with |f32(psort[j] - lab)| <= 0.5 (f32 predicate identical to the
    reference).  searchsorted gives a 1-ulp-accurate seed; refine locally."""
    lab = lab.astype(np.float32)
    n = len(psort)
    lo = np.searchsorted(psort, (lab - np.float32(0.5)).astype(np.float32),
                         side="left").astype(np.int64)
    hi = np.searchsorted(psort, (lab + np.float32(0.5)).astype(np.float32),
                         side="right").astype(np.int64)

    def pred(idx):
        idxc = np.clip(idx, 0, n - 1)
        v = np.abs((psort[idxc] - lab).astype(np.float32)) <= np.float32(0.5)
        return v & (idx >= 0) & (idx < n)

    for _ in range(3):
        lo = np.where(pred(lo - 1), lo - 1, lo)       # extend left
    for _ in range(3):
        shrink = ~pred(lo) & (lo < hi)
        lo = np.where(shrink, lo + 1, lo)             # shrink left
    for _ in range(3):
        hi = np.where(pred(hi), hi + 1, hi)           # extend right
    for _ in range(3):
        shrink = ~pred(hi - 1) & (hi > lo)
        hi = np.where(shrink, hi - 1, hi)             # shrink right
    hi = np.maximum(hi, lo)
    return lo, hi


def _plan(labels, proto):
    labels = np.asarray(labels, np.float32)
    proto = np.asarray(proto, np.float32)
    colperm = np.argsort(proto, kind="stable")
    psort = proto[colperm]
    roworder = np.argsort(labels, kind="stable")
    lo, hi = _exact_bounds(psort, labels[roworder])

    glo = lo.reshape(NG, 128)
    ghi = hi.reshape(NG, 128)
    gBLO = (glo.min(axis=1) // ALIGN) * ALIGN
    gBHI = -(-ghi.max(axis=1) // ALIGN) * ALIGN
    gW = np.maximum(gBHI - gBLO, ALIGN)

    rank = np.argsort(-gW, kind="stable")
    W_t = np.zeros(RT, np.int64)
    assign = np.zeros((NCORES, RT), np.int64)
    for t in range(RT):
        grp = rank[NCORES * t:NCORES * (t + 1)]
        W_t[t] = gW[grp].max()
        assign[:, t] = grp

    ILO_t = np.zeros(RT, np.int64)
    IHI_t = np.zeros(RT, np.int64)
    a_loc = np.zeros((NCORES, RT, 128), np.int64)
    b_loc = np.zeros((NCORES, RT, 128), np.int64)
    for t in range(RT):
        ilo, ihi = 0, 1 << 40
        for c in range(NCORES):
            g = assign[c, t]
            a = glo[g] - gBLO[g]
            b = ghi[g] - gBLO[g]
            a_loc[c, t] = a
            b_loc[c, t] = b
            ilo = max(ilo, -(-a.max() // ALIGN) * ALIGN)
            ihi = min(ihi, (b.min() // ALIGN) * ALIGN)
        ihi = min(ihi, int(W_t[t]))
        ilo = min(ilo, int(W_t[t]))
        if ihi < ilo:
            ihi = ilo
        ILO_t[t], IHI_t[t] = ilo, ihi

    # op list (tile-order): (tile, kind, width); kind 0=interior,1=left,2=right
    ops = []
    for t in range(RT):
        if IHI_t[t] > ILO_t[t]:
            ops.append((t, 0, int(IHI_t[t] - ILO_t[t])))
        if ILO_t[t] > 0:
            ops.append((t, 1, int(ILO_t[t])))
        if W_t[t] > IHI_t[t]:
            ops.append((t, 2, int(W_t[t] - IHI_t[t])))
    rampw = max(
        [8] + [w for (_, k, w) in ops if k != 0]
    )
    rampw = -(-rampw // ALIGN) * ALIGN
    return dict(colperm=colperm, roworder=roworder, gBLO=gBLO,
                W_t=W_t, assign=assign, ILO_t=ILO_t, IHI_t=IHI_t,
                a_loc=a_loc, b_loc=b_loc, ops=ops, rampw=rampw)


# --------------------------------------------------------------- device
def _get_bass(pl):
    key = ("v2", tuple(pl["W_t"]), tuple(pl["ILO_t"]), tuple(pl["IHI_t"]),
           pl["rampw"])
    if key in _cache:
        return _cache[key]
    _ensure_path()
    import concourse.bacc as bacc
    import concourse.mybir as mybir

    mm_op, sc_op = _register_ops()
    f32 = mybir.dt.float32
    bf16 = mybir.dt.bfloat16
    f16 = mybir.dt.float16

    W_t = [int(x) for x in pl["W_t"]]
    ILO = [int(x) for x in pl["ILO_t"]]
    IHI = [int(x) for x in pl["IHI_t"]]
    ops = pl["ops"]
    NOPS = len(ops)
    SUMW = sum(W_t)
    RAMPW = int(pl["rampw"])
    MAXW = max(W_t)
    SCRW = MAXW + 4 * NOPS + 8

    nc = bacc.Bacc(
        "TRN2", target_bir_lowering=False, debug=False, num_devices=NCORES
    )
    d_ap = nc.dram_tensor("d", [128, SUMW], bf16, kind="ExternalInput").ap()
    cons_ap = nc.dram_tensor("cons", [128, 2 * RT], f32,
                             kind="ExternalInput").ap()
    ramp_ap = nc.dram_tensor("ramp", [128, RAMPW], f16,
                             kind="ExternalInput").ap()
    out_ap = nc.dram_tensor("rmin", [128, 4 * NOPS], bf16,
                            kind="ExternalOutput").ap()

    dbig = nc.alloc_sbuf_tensor("dbig_t", [128, SUMW], bf16).ap()
    scr = nc.alloc_sbuf_tensor("scr_t", [128, SCRW], bf16).ap()
    cons = nc.alloc_sbuf_tensor("cons_t", [128, 2 * RT], f32).ap()
    ramp = nc.alloc_sbuf_tensor("ramp_t", [128, RAMPW], f16).ap()

    d_sems = [nc.alloc_semaphore(f"d{t}") for t in range(RT)]
    aux_sem = nc.alloc_semaphore("aux")
    dve_sem = nc.alloc_semaphore("dve")
    out_sem = nc.alloc_semaphore("out")

    off_t = np.concatenate([[0], np.cumsum(W_t)])

    with nc.Block() as block:

        @block.sync
        def _(sync):
            sync.dma_start(cons[:], cons_ap[:]).then_inc(aux_sem, 16)
            sync.dma_start(ramp[:], ramp_ap[:]).then_inc(aux_sem, 16)
            for t in range(RT):
                sync.dma_start(
                    dbig[:, int(off_t[t]):int(off_t[t + 1])],
                    d_ap[:, int(off_t[t]):int(off_t[t + 1])],
                ).then_inc(d_sems[t], 16)
            sync.wait_ge(dve_sem, NOPS)
            sync.dma_start(
                out_ap[:], scr[:, SCRW - 4 * NOPS:SCRW]
            ).then_inc(out_sem, 16)
            sync.wait_ge(out_sem, 16)
            all_sems = sorted(
                s.num for s in [*d_sems, aux_sem, dve_sem, out_sem]
            )
            lo = prev = all_sems[0]
            for n in all_sems[1:] + [None]:
                if n is not None and n == prev + 1:
                    prev = n
                    continue
                sync.sem_clear(range(lo, prev + 1))
                if n is not None:
                    lo = prev = n

        @block.vector
        def _(vector):
            waited_aux = False
            waited_tile = -1
            for k, (t, kind, w) in enumerate(ops):
                if waited_tile < t:
                    vector.wait_ge(d_sems[t], 16)
                    waited_tile = t
                if kind != 0 and not waited_aux:
                    vector.wait_ge(aux_sem, 32)
                    waited_aux = True
                end = SCRW - 4 * k
                o = int(off_t[t])
                if kind == 0:
                    inst = nc.vector._custom_dve(
                        sc_op,
                        out=scr[:, end - w:end],
                        in0=dbig[:, o + ILO[t]:o + IHI[t]],
                        imm2=float(BIG),
                    )
                    inst.ins.perf_max = 3
                elif kind == 1:
                    inst = nc.vector._custom_dve(
                        mm_op,
                        out=scr[:, end - w:end],
                        in0=dbig[:, o:o + w],
                        in1=ramp[:, :w],
                        s0=cons[:, t:t + 1],
                        s1=float(HALF_W),
                        imm2=float(BIG),
                    )
                    inst.ins.perf_max = 1
                else:
                    inst = nc.vector._custom_dve(
                        mm_op,
                        out=scr[:, end - w:end],
                        in0=dbig[:, o + IHI[t]:o + W_t[t]],
                        in1=ramp[:, :w],
                        s0=cons[:, RT + t:RT + t + 1],
                        s1=float(HALF_W),
                        imm2=float(BIG),
                    )
                    inst.ins.perf_max = 1
                inst.then_inc(dve_sem, 1)

    nc.compile()
    _cache[key] = nc
    return nc


# --------------------------------------------------------------- staging
def _stage(min_distances, labels, proto_classes, pl):
    import ml_dtypes

    bf16 = ml_dtypes.bfloat16
    d = np.asarray(min_distances, np.float32)
    dcols = np.ascontiguousarray(d[:, pl["colperm"]])
    W_t = pl["W_t"]
    NOPS = len(pl["ops"])
    rampw = int(pl["rampw"])
    ramp = np.broadcast_to(
        np.arange(rampw, dtype=np.float16)[None, :], (128, rampw)
    )
    ramp = np.ascontiguousarray(ramp)

    in_maps = []
    for c in range(NCORES):
        segs = []
        consL = np.zeros((128, RT), np.float32)
        consR = np.zeros((128, RT), np.float32)
        for t in range(RT):
            g = int(pl["assign"][c, t])
            rows = pl["roworder"][128 * g:128 * (g + 1)]
            blo = int(pl["gBLO"][g])
            w = int(W_t[t])
            seg = np.full((128, w), BIG, np.float32)
            real = max(0, min(w, P - blo))
            seg[:, :real] = dcols[rows, blo:blo + real]
            segs.append(seg)
            a = pl["a_loc"][c, t].astype(np.float32)
            bb = (pl["b_loc"][c, t] - pl["IHI_t"][t]).astype(np.float32)
            consL[:, t] = a + np.float32(HALF_W)
            consR[:, t] = bb - np.float32(1.0) - np.float32(HALF_W)
        dcat = np.concatenate(segs, axis=1).astype(bf16)
        cons = np.concatenate([consL, consR], axis=1)
        in_maps.append(
            {"d": np.ascontiguousarray(dcat), "cons": cons, "ramp": ramp}
        )
    return in_maps


def kernel(min_distances, labels, proto_classes):
    _ensure_path()
    pl = _plan(labels, proto_classes)
    nc = _get_bass(pl)
    from concourse.bass_utils import run_bass_kernel_spmd

    in_maps = _stage(min_distances, labels, proto_classes, pl)
    res = run_bass_kernel_spmd(
        nc, in_maps, core_ids=list(range(NCORES))
    ).results

    ops = pl["ops"]
    NOPS = len(ops)
    loss_rows = np.zeros(B, np.float64)
    acc = np.full((NCORES, RT, 128), np.float32(BIG), np.float32)
    for c in range(NCORES):
        r = np.asarray(res[c]["rmin"]).astype(np.float32)  # [128, 4*NOPS]
        for k, (t, kind, w) in enumerate(ops):
            # op k's final value: scratch col (SCRW - 4k - 1) -> gather-local
            col = 4 * (NOPS - k) - 1
            acc[c, t] = np.minimum(acc[c, t], r[:, col])
    for c in range(NCORES):
        for t in range(RT):
            g = int(pl["assign"][c, t])
            rows = pl["roworder"][128 * g:128 * (g + 1)]
            dmin = acc[c, t]
            lr = np.where(
                dmin >= np.float32(BIG / 2),
                np.float32(128.0),
                (MAX_DIST - (MAX_DIST - dmin).astype(np.float32)).astype(
                    np.float32
                ),
            )
            loss_rows[rows] = lr
    return np.array(loss_rows.mean(dtype=np.float64), dtype=np.float32)


# revision 6
# speedup vs baseline: 1.1325x; 1.0226x over previous
"""Trainium2 Bass kernel for nn_ClusterLoss_Regr (topk_masking) — v2.

Computes  mean_b(128 - max_p((128 - d[b,p]) * [|proto[p] - label[b]| <= 0.5]))
for d: [8192, 4096] f32, labels: [8192] f32, proto: [4096] f32 -> scalar f32.

v2 design ("sorted staircase"):
  - Host sorts columns by proto value and rows by label.  Each row's mask is
    then one contiguous column range [lo_r, hi_r).  Rows are grouped into 64
    groups of 128 consecutive sorted rows; groups are rank-bucketed by their
    union mask width so the 8 groups of tile-index t have near-equal widths
    (uniform compile-time shapes across the 8 SPMD cores).  Columns outside
    a tile's union range are never staged or read (~21% HBM traffic cut).
  - Per 128-row tile the union range splits into an all-masked INTERIOR
    (every row of the tile wants these columns) and two narrow boundary
    BANDS where the mask varies by row.
  - INTERIOR: a custom single-source running-min DVE op with hand-written
    2X_1PORT / 2X_2PORT / 4X_2PORT uop programs -> 4 elem/lane/cycle.
    No mask needed, no proto data on device at all.
  - BANDS: the masked-min op compares an f16 index ramp against per-row
    range constants (|j - c0| <= 512 encodes j >= a_r or j < b_r), 2X mode.
    Mask bounds are computed EXACTLY on host (f32 predicate refinement).
  - All DVE ops write into one shared scratch buffer with descending end
    offsets (step 4) so every op's final running-min lands in a contiguous
    window -> ONE tail gather DMA instead of one per op.
  - d staged as bf16 (measured: plain bf16 HWDGE sustains ~395 GB/s/core;
    fp8 cast-DMA is write-side-bound and no faster).
Host: decode rowmins, map through the row permutation, mean in f64.
"""

import numpy as np

B, P = 8192, 4096
NCORES = 8
BSH = B // NCORES      # 1024 rows per core
RT = BSH // 128        # 8 row-tiles of 128 rows
NG = B // 128          # 64 row groups
ALIGN = 8
MAX_DIST = np.float32(128.0)
BIG = 2.0              # > max(d)=1.0
HALF_W = 512.0         # band compare halfwidth (band widths < 512)

_cache: dict = {}


def _ensure_path():
    try:
        import concourse.bass  # noqa: F401
    except ImportError:
        import sys

        for p in ("/opt/trn_rl_repo",):
            if p not in sys.path:
                sys.path.insert(0, p)


# --------------------------------------------------------------- DVE ops
def _build_maskmin_2x_uops():
    """2X_1PORT program for the masked-min op (see spec in _register_ops):
    v = select(C1 >= |Src1 - C0|, Src0, C2); acc = min(acc, v); acc streamed
    to both write halves.  Identical to the proven v1 program."""
    from concourse.dve_uop import (
        AluInp,
        AluOp,
        DelayInp,
        InpSel,
        OutPath,
        OutSel,
        Trigger,
        UopConfig,
        UopDpConfig,
    )

    ENABLE = 1
    P_AD = AluInp.PREV_ALU_OUT
    CUR = AluInp.CURR_ALU_OUT
    D = [
        AluInp.PREV_DELAY_0,
        AluInp.PREV_DELAY_1,
        AluInp.PREV_DELAY_2,
        AluInp.PREV_DELAY_3,
        AluInp.PREV_DELAY_4,
        AluInp.PREV_DELAY_5,
    ]
    SRC_DONE = (Trigger.SRC_TENSOR_DONE, Trigger.NONE, Trigger.NONE)
    COUNT_ONCE = (Trigger.COUNT, Trigger.NONE, Trigger.NONE)

    def wire_inputs(u):
        u.enable_input(InpSel.SRC_0, 0)      # input 0 -> b0's PREV_ALU_OUT
        u.enable_input(InpSel.SRC_1, 1)      # lane 0
        u.enable_input(InpSel.CONST_0, 2)    # lane 1: center
        u.enable_input(InpSel.CONST_1, 3)    # lane 2: halfwidth
        u.enable_input(InpSel.CONST_2, 4)    # lane 3: BIG
        u.enable_input(InpSel.SRC_0_HI, 5)   # lane 4
        u.enable_input(InpSel.SRC_1_HI, 6)   # lane 5

    def steady_blocks():
        dp = [UopDpConfig() for _ in range(8)]
        for i in range(8):
            dp[i].pass_through_delay(1, 2, 3, 5)
            if i not in (0, 3):
                dp[i].pass_through_delay(0)
            if i != 6:
                dp[i].pass_through_delay(4)
        dp[0].enable_alu(AluOp.ABSOLUTE_DIFF, D[0], D[1])
        dp[0].enable_delay_from_src(DelayInp.PREV_ALU_OUT, 0)  # Src0
        dp[1].enable_alu(AluOp.IS_GE, D[2], P_AD)
        dp[2].enable_alu(AluOp.SELECT, D[3], D[0])
        dp[3].enable_alu(AluOp.ABSOLUTE_DIFF, D[5], D[1])
        dp[3].enable_delay_from_src(DelayInp.PREV_ALU_OUT, 0)  # v_lo
        dp[4].enable_alu(AluOp.IS_GE, D[2], P_AD)
        dp[5].enable_alu(AluOp.SELECT, D[3], D[4])
        dp[6].enable_alu(AluOp.MIN, D[0], P_AD)
        dp[7].enable_alu(AluOp.MIN, CUR, P_AD)
        return dp

    sd = UopConfig(trigger=COUNT_ONCE, repeat_count=1, next_uop=(1, 0, 0))
    wire_inputs(sd)
    dp = steady_blocks()
    dp[7] = UopDpConfig()
    dp[7].pass_through_delay(0, 1, 2, 3, 4, 5)
    dp[7].enable_alu(AluOp.BYPASS, D[3], D[3])
    sd.datapath_config = dp

    st = UopConfig(trigger=SRC_DONE, require_inp0=ENABLE, require_inp1=ENABLE)
    wire_inputs(st)
    st.datapath_config = steady_blocks()
    st.enable_output(OutSel.ALU_OUT, OutPath.WR0_LO)
    st.enable_output(OutSel.ALU_OUT, OutPath.WR0_HI)
    return [sd, st]


def _build_scan_uops(nports):
    """Pure running-min programs.  nports=1 -> 2X_1PORT, 2 -> 2X_2PORT,
    4 -> 4X_2PORT.  Lanes: 0=SRC_0, 1=SRC_0_HI, 2=SRC_1, 3=SRC_1_HI,
    4=CONST_2 (seed).  state0 seeds blk7's out-flop; state1:
    acc = min(acc, min(elems)), streamed to the write ports."""
    from concourse.dve_uop import (
        AluInp,
        AluOp,
        DelayInp,
        InpSel,
        OutPath,
        OutSel,
        Trigger,
        UopConfig,
        UopDpConfig,
    )

    P_AD = AluInp.PREV_ALU_OUT
    CUR = AluInp.CURR_ALU_OUT
    D = [
        AluInp.PREV_DELAY_0,
        AluInp.PREV_DELAY_1,
        AluInp.PREV_DELAY_2,
        AluInp.PREV_DELAY_3,
        AluInp.PREV_DELAY_4,
        AluInp.PREV_DELAY_5,
    ]
    COUNT_ONCE = (Trigger.COUNT, Trigger.NONE, Trigger.NONE)
    SRC_DONE = (Trigger.SRC_TENSOR_DONE, Trigger.NONE, Trigger.NONE)

    def wire_inputs(u):
        u.enable_input(InpSel.SRC_0, 0)
        u.enable_input(InpSel.SRC_0_HI, 1)
        if nports >= 2:
            u.enable_input(InpSel.SRC_1, 2)
        if nports == 4:
            u.enable_input(InpSel.SRC_1_HI, 3)
        u.enable_input(InpSel.CONST_2, 4)

    def passthroughs():
        dp = [UopDpConfig() for _ in range(8)]
        for i in range(8):
            dp[i].pass_through_delay(0, 1, 2, 3, 4, 5)
        return dp

    def steady():
        dp = [UopDpConfig() for _ in range(8)]
        if nports == 4:
            dp[0].pass_through_delay(1, 2, 3, 4, 5)
            dp[0].enable_alu(AluOp.MIN, P_AD, D[0])        # m0 = min(A, B)
            dp[1].pass_through_delay(1, 2, 3, 4, 5)
            dp[1].enable_alu(AluOp.MIN, D[1], D[2])        # m1 = min(C, D)
            dp[1].enable_delay_from_src(DelayInp.PREV_ALU_OUT, 0)  # d0 <- m0
            dp[2].pass_through_delay(1, 2, 3, 4, 5)
            dp[2].enable_alu(AluOp.MIN, P_AD, D[0])        # m = min(m1, m0)
            first_bypass = 3
        elif nports == 2:
            dp[0].pass_through_delay(0, 1, 3, 4, 5)
            dp[0].enable_alu(AluOp.MIN, P_AD, D[1])        # min(A, C)
            first_bypass = 1
        else:
            dp[0].pass_through_delay(1, 2, 3, 4, 5)
            dp[0].enable_alu(AluOp.MIN, P_AD, D[0])        # min(A, B)
            first_bypass = 1
        for i in range(first_bypass, 7):
            dp[i].pass_through_delay(0, 1, 2, 3, 4, 5)
            dp[i].enable_alu(AluOp.BYPASS, P_AD, P_AD)
        dp[7].pass_through_delay(0, 1, 2, 3, 4, 5)
        dp[7].enable_alu(AluOp.MIN, CUR, P_AD)             # acc
        return dp

    sd = UopConfig(trigger=COUNT_ONCE, repeat_count=1, next_uop=(1, 0, 0))
    wire_inputs(sd)
    dp = passthroughs()
    dp[7] = UopDpConfig()
    dp[7].pass_through_delay(0, 1, 2, 4, 5)
    dp[7].enable_alu(AluOp.BYPASS, D[3], D[3])             # seed <- BIG
    sd.datapath_config = dp

    st = UopConfig(
        trigger=SRC_DONE,
        require_inp0=1,
        require_inp1=1 if nports >= 2 else 0,
    )
    wire_inputs(st)
    st.datapath_config = steady()
    st.enable_output(OutSel.ALU_OUT, OutPath.WR0_LO)
    st.enable_output(OutSel.ALU_OUT, OutPath.WR0_HI)
    if nports >= 2:
        st.enable_output(OutSel.ALU_OUT, OutPath.WR1_LO)
        st.enable_output(OutSel.ALU_OUT, OutPath.WR1_HI)
    return [sd, st]


def _register_ops():
    """Register the masked-min (2X) and pure-min-scan (4X) ops. Idempotent."""
    from concourse import dve_ops
    from concourse.dve_spec import (
        C0,
        C1,
        C2,
        AluOp,
        Bin,
        Spec,
        Src0,
        Src1,
        lower,
        scan,
        select,
    )
    from concourse.dve_uop import DveOpSpec

    def _make(name, spec, build_variants, perf_max, rd1_en):
        for op in dve_ops.OPS:
            if op.name == name:
                return op

        class _Op(dve_ops.DveOp):
            def compile(self, ver):
                key = (self.name, ver)
                if (r := dve_ops._COMPILE_CACHE.get(key)) is not None:
                    return r
                uops = lower(self.spec, ver=ver)
                variants = {}
                if ver == "v3":
                    variants = build_variants()
                    for v in variants.values():
                        assert len(v) == len(uops), (name, len(v), len(uops))
                result = DveOpSpec(
                    name=self.name,
                    opcode=dve_ops.get_dve_sub_opcode(self.name),
                    uops=uops,
                    uops_2x=variants.get("2x"),
                    uops_2x_2p=variants.get("2x_2p"),
                    uops_4x=variants.get("4x"),
                    perf_max=perf_max if variants else 0,
                    rd1_en=rd1_en,
                )
                dve_ops._COMPILE_CACHE[key] = result
                return result

        shas: dict = {}
        op = _Op(name, spec, subdim=False, uops_sha=shas)
        dve_ops.OPS.append(op)
        row = dve_ops._CUSTOM_DVE_ROW_BASE + len(dve_ops.OPS) - 1
        dve_ops._SUB_OPCODE_FOR_NAME[name] = row
        dve_ops.CUSTOM_DVE_SPECS[name] = spec
        for ver in ("v3", "v4"):
            shas[ver] = op.compile(ver).sha(ver) if ver == "v3" else ""
        return op

    def _mm_ref(in0, in1, s0, s1, imm2):
        mask = np.abs(in1.astype(np.float32) - np.asarray(s0, np.float32)) <= (
            np.float32(s1)
        )
        o = np.where(mask, in0.astype(np.float32), np.float32(imm2)).astype(
            np.float32
        )
        return np.minimum.accumulate(o, axis=-1)

    mm_spec = Spec(
        body=scan(
            AluOp.MIN,
            select(C1 >= Bin(AluOp.ABSOLUTE_DIFF, Src1, C0), Src0, C2),
            init=C2,
        ),
        reference=_mm_ref,
    )
    mm = _make(
        "CLUSTER_MASKMIN_ANT",
        mm_spec,
        lambda: {"2x": _build_maskmin_2x_uops()},
        perf_max=1,
        rd1_en=True,
    )

    def _sc_ref(in0, in1, s0, s1, imm2):
        o = np.minimum.accumulate(in0.astype(np.float32), axis=-1)
        return np.minimum(o, np.float32(imm2))

    sc_spec = Spec(body=scan(AluOp.MIN, Src0, init=C2), reference=_sc_ref)
    sc = _make(
        "PUREMIN_SCAN_ANT",
        sc_spec,
        lambda: {
            "2x": _build_scan_uops(1),
            "2x_2p": _build_scan_uops(2),
            "4x": _build_scan_uops(4),
        },
        perf_max=3,
        rd1_en=False,
    )
    return mm, sc


# --------------------------------------------------------------- host plan
def _exact_bounds(psort, lab):
    """Exact contiguous mask range per row: first/last sorted-proto index j
    with |f32(psort[j] - lab)| <= 0.5 (f32 predicate identical to the
    reference).  searchsorted gives a 1-ulp-accurate seed; refine locally."""
    lab = lab.astype(np.float32)
    n = len(psort)
    lo = np.searchsorted(psort, (lab - np.float32(0.5)).astype(np.float32),
                         side="left").astype(np.int64)
    hi = np.searchsorted(psort, (lab + np.float32(0.5)).astype(np.float32),
                         side="right").astype(np.int64)

    def pred(idx):
        idxc = np.clip(idx, 0, n - 1)
        v = np.abs((psort[idxc] - lab).astype(np.float32)) <= np.float32(0.5)
        return v & (idx >= 0) & (idx < n)

    for _ in range(3):
        lo = np.where(pred(lo - 1), lo - 1, lo)       # extend left
    for _ in range(3):
        shrink = ~pred(lo) & (lo < hi)
        lo = np.where(shrink, lo + 1, lo)             # shrink left
    for _ in range(3):
        hi = np.where(pred(hi), hi + 1, hi)           # extend right
    for _ in range(3):
        shrink = ~pred(hi - 1) & (hi > lo)
        hi = np.where(shrink, hi - 1, hi)             # shrink right
    hi = np.maximum(hi, lo)
    return lo, hi


def _plan(labels, proto):
    labels = np.asarray(labels, np.float32)
    proto = np.asarray(proto, np.float32)
    colperm = np.argsort(proto, kind="stable")
    psort = proto[colperm]
    roworder = np.argsort(labels, kind="stable")
    lo, hi = _exact_bounds(psort, labels[roworder])

    glo = lo.reshape(NG, 128)
    ghi = hi.reshape(NG, 128)
    gBLO = (glo.min(axis=1) // ALIGN) * ALIGN
    gBHI = -(-ghi.max(axis=1) // ALIGN) * ALIGN
    gW = np.maximum(gBHI - gBLO, ALIGN)

    rank = np.argsort(-gW, kind="stable")
    W_t = np.zeros(RT, np.int64)
    assign = np.zeros((NCORES, RT), np.int64)
    for t in range(RT):
        grp = rank[NCORES * t:NCORES * (t + 1)]
        W_t[t] = gW[grp].max()
        assign[:, t] = grp

    ILO_t = np.zeros(RT, np.int64)
    IHI_t = np.zeros(RT, np.int64)
    a_loc = np.zeros((NCORES, RT, 128), np.int64)
    b_loc = np.zeros((NCORES, RT, 128), np.int64)
    for t in range(RT):
        ilo, ihi = 0, 1 << 40
        for c in range(NCORES):
            g = assign[c, t]
            a = glo[g] - gBLO[g]
            b = ghi[g] - gBLO[g]
            a_loc[c, t] = a
            b_loc[c, t] = b
            ilo = max(ilo, -(-a.max() // ALIGN) * ALIGN)
            ihi = min(ihi, (b.min() // ALIGN) * ALIGN)
        ihi = min(ihi, int(W_t[t]))
        ilo = min(ilo, int(W_t[t]))
        if ihi < ilo:
            ihi = ilo
        ILO_t[t], IHI_t[t] = ilo, ihi

    # per-tile DMA split point (multiple of ALIGN, inside [ILO, IHI])
    MID_t = np.zeros(RT, np.int64)
    for t in range(RT):
        m = (int(W_t[t]) // 2 // ALIGN) * ALIGN
        MID_t[t] = min(max(m, int(ILO_t[t])), int(IHI_t[t]))
    # op list (half-gated order): (tile, kind, width, half)
    # kind 0=interior lo half, 3=interior hi half, 1=left band, 2=right band
    ops = []
    for t in range(RT):
        if MID_t[t] > ILO_t[t]:
            ops.append((t, 0, int(MID_t[t] - ILO_t[t]), 0))
        if ILO_t[t] > 0:
            ops.append((t, 1, int(ILO_t[t]), 0))
        if IHI_t[t] > MID_t[t]:
            ops.append((t, 3, int(IHI_t[t] - MID_t[t]), 1))
        if W_t[t] > IHI_t[t]:
            ops.append((t, 2, int(W_t[t] - IHI_t[t]), 1))
    rampw = max(
        [8] + [w for (_, k, w, _h) in ops if k in (1, 2)]
    )
    rampw = -(-rampw // ALIGN) * ALIGN
    return dict(colperm=colperm, roworder=roworder, gBLO=gBLO,
                W_t=W_t, assign=assign, ILO_t=ILO_t, IHI_t=IHI_t,
                MID_t=MID_t, a_loc=a_loc, b_loc=b_loc, ops=ops, rampw=rampw)


# --------------------------------------------------------------- device
def _get_bass(pl):
    key = ("v21", tuple(pl["W_t"]), tuple(pl["ILO_t"]), tuple(pl["IHI_t"]),
           tuple(pl["MID_t"]), pl["rampw"])
    if key in _cache:
        return _cache[key]
    _ensure_path()
    import concourse.bacc as bacc
    import concourse.mybir as mybir

    mm_op, sc_op = _register_ops()
    bf16 = mybir.dt.bfloat16
    f16 = mybir.dt.float16

    W_t = [int(x) for x in pl["W_t"]]
    ILO = [int(x) for x in pl["ILO_t"]]
    IHI = [int(x) for x in pl["IHI_t"]]
    MID = [int(x) for x in pl["MID_t"]]
    ops = pl["ops"]
    NOPS = len(ops)
    SUMW = sum(W_t)
    RAMPW = int(pl["rampw"])
    AUXW = RAMPW + 4 * RT  # trailing 2*RT f32 consts stored as f16 pairs
    MAXW = max(W_t)
    SCRW = MAXW + 4 * NOPS + 8

    nc = bacc.Bacc(
        "TRN2", target_bir_lowering=False, debug=False, num_devices=NCORES
    )
    d_ap = nc.dram_tensor("d", [128, SUMW], bf16, kind="ExternalInput").ap()
    aux_ap = nc.dram_tensor("aux", [128, AUXW], f16, kind="ExternalInput").ap()
    out_ap = nc.dram_tensor("rmin", [128, 4 * NOPS], bf16,
                            kind="ExternalOutput").ap()

    dbig = nc.alloc_sbuf_tensor("dbig_t", [128, SUMW], bf16).ap()
    scr = nc.alloc_sbuf_tensor("scr_t", [128, SCRW], bf16).ap()
    aux = nc.alloc_sbuf_tensor("aux_t", [128, AUXW], f16).ap()
    ramp = aux[:, :RAMPW]
    aux32 = aux.bitcast(mybir.dt.float32)  # [128, AUXW // 2]
    consL = aux32[:, RAMPW // 2:RAMPW // 2 + RT]
    consR = aux32[:, RAMPW // 2 + RT:RAMPW // 2 + 2 * RT]

    d_sems = [nc.alloc_semaphore(f"d{t}h{h}") for t in range(RT)
              for h in range(2)]
    aux_sem = nc.alloc_semaphore("aux")
    dve_sem = nc.alloc_semaphore("dve")
    out_sem = nc.alloc_semaphore("out")

    off_t = np.concatenate([[0], np.cumsum(W_t)])
    # DMA plan: (tile, half) halves split at MID; aux inserted after tile 1
    dma_list = []
    for t in range(RT):
        o = int(off_t[t])
        dma_list.append((t, 0, o, o + MID[t]))
        dma_list.append((t, 1, o + MID[t], o + W_t[t]))

    with nc.Block() as block:

        @block.sync
        def _(sync):
            for i, (t, h, lo, hi) in enumerate(dma_list):
                if hi > lo:
                    # every op's gating half is nonempty by construction;
                    # an empty half has no waiter, so just skip its DMA
                    sync.dma_start(
                        dbig[:, lo:hi], d_ap[:, lo:hi]
                    ).then_inc(d_sems[2 * t + h], 16)
                if i == 3:
                    sync.dma_start(aux[:], aux_ap[:]).then_inc(aux_sem, 16)
            sync.wait_ge(dve_sem, NOPS)
            sync.dma_start(
                out_ap[:], scr[:, SCRW - 4 * NOPS:SCRW]
            ).then_inc(out_sem, 16)
            sync.wait_ge(out_sem, 16)
            all_sems = sorted(
                s.num for s in [*d_sems, aux_sem, dve_sem, out_sem]
            )
            lo = prev = all_sems[0]
            for n in all_sems[1:] + [None]:
                if n is not None and n == prev + 1:
                    prev = n
                    continue
                sync.sem_clear(range(lo, prev + 1))
                if n is not None:
                    lo = prev = n

        @block.vector
        def _(vector):
            waited_aux = False
            waited = set()
            for k, (t, kind, w, h) in enumerate(ops):
                if (t, h) not in waited:
                    vector.wait_ge(d_sems[2 * t + h], 16)
                    waited.add((t, h))
                if kind in (1, 2) and not waited_aux:
                    vector.wait_ge(aux_sem, 16)
                    waited_aux = True
                end = SCRW - 4 * k
                o = int(off_t[t])
                if kind == 0:
                    inst = nc.vector._custom_dve(
                        sc_op,
                        out=scr[:, end - w:end],
                        in0=dbig[:, o + ILO[t]:o + MID[t]],
                        imm2=float(BIG),
                    )
                    inst.ins.perf_max = 3
                elif kind == 3:
                    inst = nc.vector._custom_dve(
                        sc_op,
                        out=scr[:, end - w:end],
                        in0=dbig[:, o + MID[t]:o + IHI[t]],
                        imm2=float(BIG),
                    )
                    inst.ins.perf_max = 3
                elif kind == 1:
                    inst = nc.vector._custom_dve(
                        mm_op,
                        out=scr[:, end - w:end],
                        in0=dbig[:, o:o + w],
                        in1=ramp[:, :w],
                        s0=consL[:, t:t + 1],
                        s1=float(HALF_W),
                        imm2=float(BIG),
                    )
                    inst.ins.perf_max = 1
                else:
                    inst = nc.vector._custom_dve(
                        mm_op,
                        out=scr[:, end - w:end],
                        in0=dbig[:, o + IHI[t]:o + W_t[t]],
                        in1=ramp[:, :w],
                        s0=consR[:, t:t + 1],
                        s1=float(HALF_W),
                        imm2=float(BIG),
                    )
                    inst.ins.perf_max = 1
                inst.then_inc(dve_sem, 1)

    nc.compile()
    _cache[key] = nc
    return nc


# --------------------------------------------------------------- staging
def _stage(min_distances, labels, proto_classes, pl):
    import ml_dtypes

    bf16 = ml_dtypes.bfloat16
    d = np.asarray(min_distances, np.float32)
    dcols = np.ascontiguousarray(d[:, pl["colperm"]])
    W_t = pl["W_t"]
    rampw = int(pl["rampw"])
    ramp = np.arange(rampw, dtype=np.float16)

    in_maps = []
    for c in range(NCORES):
        segs = []
        aux = np.zeros((128, rampw + 4 * RT), np.float16)
        aux[:, :rampw] = ramp[None, :]
        consf32 = aux[:, rampw:].view(np.float32)  # [128, 2*RT]
        for t in range(RT):
            g = int(pl["assign"][c, t])
            rows = pl["roworder"][128 * g:128 * (g + 1)]
            blo = int(pl["gBLO"][g])
            w = int(W_t[t])
            seg = np.full((128, w), BIG, np.float32)
            real = max(0, min(w, P - blo))
            seg[:, :real] = dcols[rows, blo:blo + real]
            segs.append(seg)
            a = pl["a_loc"][c, t].astype(np.float32)
            bb = (pl["b_loc"][c, t] - pl["IHI_t"][t]).astype(np.float32)
            consf32[:, t] = a + np.float32(HALF_W)
            consf32[:, RT + t] = bb - np.float32(1.0) - np.float32(HALF_W)
        dcat = np.concatenate(segs, axis=1).astype(bf16)
        in_maps.append(
            {"d": np.ascontiguousarray(dcat), "aux": np.ascontiguousarray(aux)}
        )
    return in_maps


def kernel(min_distances, labels, proto_classes):
    _ensure_path()
    pl = _plan(labels, proto_classes)
    nc = _get_bass(pl)
    from concourse.bass_utils import run_bass_kernel_spmd

    in_maps = _stage(min_distances, labels, proto_classes, pl)
    res = run_bass_kernel_spmd(
        nc, in_maps, core_ids=list(range(NCORES))
    ).results

    ops = pl["ops"]
    NOPS = len(ops)
    loss_rows = np.zeros(B, np.float64)
    acc = np.full((NCORES, RT, 128), np.float32(BIG), np.float32)
    for c in range(NCORES):
        r = np.asarray(res[c]["rmin"]).astype(np.float32)  # [128, 4*NOPS]
        for k, (t, kind, w, h) in enumerate(ops):
            # op k's final value: scratch col (SCRW - 4k - 1) -> gather-local
            col = 4 * (NOPS - k) - 1
            acc[c, t] = np.minimum(acc[c, t], r[:, col])
    for c in range(NCORES):
        for t in range(RT):
            g = int(pl["assign"][c, t])
            rows = pl["roworder"][128 * g:128 * (g + 1)]
            dmin = acc[c, t]
            lr = np.where(
                dmin >= np.float32(BIG / 2),
                np.float32(128.0),
                (MAX_DIST - (MAX_DIST - dmin).astype(np.float32)).astype(
                    np.float32
                ),
            )
            loss_rows[rows] = lr
    return np.array(loss_rows.mean(dtype=np.float64), dtype=np.float32)


# revision 7
# speedup vs baseline: 1.1413x; 1.0077x over previous
"""Trainium2 Bass kernel for nn_ClusterLoss_Regr (topk_masking) — v2.

Computes  mean_b(128 - max_p((128 - d[b,p]) * [|proto[p] - label[b]| <= 0.5]))
for d: [8192, 4096] f32, labels: [8192] f32, proto: [4096] f32 -> scalar f32.

v2 design ("sorted staircase"):
  - Host sorts columns by proto value and rows by label.  Each row's mask is
    then one contiguous column range [lo_r, hi_r).  Rows are grouped into 64
    groups of 128 consecutive sorted rows; groups are rank-bucketed by their
    union mask width so the 8 groups of tile-index t have near-equal widths
    (uniform compile-time shapes across the 8 SPMD cores).  Columns outside
    a tile's union range are never staged or read (~21% HBM traffic cut).
  - Per 128-row tile the union range splits into an all-masked INTERIOR
    (every row of the tile wants these columns) and two narrow boundary
    BANDS where the mask varies by row.
  - INTERIOR: a custom single-source running-min DVE op with hand-written
    2X_1PORT / 2X_2PORT / 4X_2PORT uop programs -> 4 elem/lane/cycle.
    No mask needed, no proto data on device at all.
  - BANDS: the masked-min op compares an f16 index ramp against per-row
    range constants (|j - c0| <= 512 encodes j >= a_r or j < b_r), 2X mode.
    Mask bounds are computed EXACTLY on host (f32 predicate refinement).
  - All DVE ops write into one shared scratch buffer with descending end
    offsets (step 4) so every op's final running-min lands in a contiguous
    window -> ONE tail gather DMA instead of one per op.
  - d staged as bf16 (measured: plain bf16 HWDGE sustains ~395 GB/s/core;
    fp8 cast-DMA is write-side-bound and no faster).
Host: decode rowmins, map through the row permutation, mean in f64.
"""

import numpy as np

B, P = 8192, 4096
NCORES = 8
BSH = B // NCORES      # 1024 rows per core
RT = BSH // 128        # 8 row-tiles of 128 rows
NG = B // 128          # 64 row groups
ALIGN = 8
MAX_DIST = np.float32(128.0)
BIG = 2.0              # > max(d)=1.0
HALF_W = 512.0         # band compare halfwidth (band widths < 512)

_cache: dict = {}


def _ensure_path():
    try:
        import concourse.bass  # noqa: F401
    except ImportError:
        import sys

        for p in ("/opt/trn_rl_repo",):
            if p not in sys.path:
                sys.path.insert(0, p)


# --------------------------------------------------------------- DVE ops
def _build_maskmin_2x_uops():
    """2X_1PORT program for the masked-min op (see spec in _register_ops):
    v = select(C1 >= |Src1 - C0|, Src0, C2); acc = min(acc, v); acc streamed
    to both write halves.  Identical to the proven v1 program."""
    from concourse.dve_uop import (
        AluInp,
        AluOp,
        DelayInp,
        InpSel,
        OutPath,
        OutSel,
        Trigger,
        UopConfig,
        UopDpConfig,
    )

    ENABLE = 1
    P_AD = AluInp.PREV_ALU_OUT
    CUR = AluInp.CURR_ALU_OUT
    D = [
        AluInp.PREV_DELAY_0,
        AluInp.PREV_DELAY_1,
        AluInp.PREV_DELAY_2,
        AluInp.PREV_DELAY_3,
        AluInp.PREV_DELAY_4,
        AluInp.PREV_DELAY_5,
    ]
    SRC_DONE = (Trigger.SRC_TENSOR_DONE, Trigger.NONE, Trigger.NONE)
    COUNT_ONCE = (Trigger.COUNT, Trigger.NONE, Trigger.NONE)

    def wire_inputs(u):
        u.enable_input(InpSel.SRC_0, 0)      # input 0 -> b0's PREV_ALU_OUT
        u.enable_input(InpSel.SRC_1, 1)      # lane 0
        u.enable_input(InpSel.CONST_0, 2)    # lane 1: center
        u.enable_input(InpSel.CONST_1, 3)    # lane 2: halfwidth
        u.enable_input(InpSel.CONST_2, 4)    # lane 3: BIG
        u.enable_input(InpSel.SRC_0_HI, 5)   # lane 4
        u.enable_input(InpSel.SRC_1_HI, 6)   # lane 5

    def steady_blocks():
        dp = [UopDpConfig() for _ in range(8)]
        for i in range(8):
            dp[i].pass_through_delay(1, 2, 3, 5)
            if i not in (0, 3):
                dp[i].pass_through_delay(0)
            if i != 6:
                dp[i].pass_through_delay(4)
        dp[0].enable_alu(AluOp.ABSOLUTE_DIFF, D[0], D[1])
        dp[0].enable_delay_from_src(DelayInp.PREV_ALU_OUT, 0)  # Src0
        dp[1].enable_alu(AluOp.IS_GE, D[2], P_AD)
        dp[2].enable_alu(AluOp.SELECT, D[3], D[0])
        dp[3].enable_alu(AluOp.ABSOLUTE_DIFF, D[5], D[1])
        dp[3].enable_delay_from_src(DelayInp.PREV_ALU_OUT, 0)  # v_lo
        dp[4].enable_alu(AluOp.IS_GE, D[2], P_AD)
        dp[5].enable_alu(AluOp.SELECT, D[3], D[4])
        dp[6].enable_alu(AluOp.MIN, D[0], P_AD)
        dp[7].enable_alu(AluOp.MIN, CUR, P_AD)
        return dp

    sd = UopConfig(trigger=COUNT_ONCE, repeat_count=1, next_uop=(1, 0, 0))
    wire_inputs(sd)
    dp = steady_blocks()
    dp[7] = UopDpConfig()
    dp[7].pass_through_delay(0, 1, 2, 3, 4, 5)
    dp[7].enable_alu(AluOp.BYPASS, D[3], D[3])
    sd.datapath_config = dp

    st = UopConfig(trigger=SRC_DONE, require_inp0=ENABLE, require_inp1=ENABLE)
    wire_inputs(st)
    st.datapath_config = steady_blocks()
    st.enable_output(OutSel.ALU_OUT, OutPath.WR0_LO)
    st.enable_output(OutSel.ALU_OUT, OutPath.WR0_HI)
    return [sd, st]


def _build_scan_uops(nports):
    """Pure running-min programs.  nports=1 -> 2X_1PORT, 2 -> 2X_2PORT,
    4 -> 4X_2PORT.  Lanes: 0=SRC_0, 1=SRC_0_HI, 2=SRC_1, 3=SRC_1_HI,
    4=CONST_2 (seed).  state0 seeds blk7's out-flop; state1:
    acc = min(acc, min(elems)), streamed to the write ports."""
    from concourse.dve_uop import (
        AluInp,
        AluOp,
        DelayInp,
        InpSel,
        OutPath,
        OutSel,
        Trigger,
        UopConfig,
        UopDpConfig,
    )

    P_AD = AluInp.PREV_ALU_OUT
    CUR = AluInp.CURR_ALU_OUT
    D = [
        AluInp.PREV_DELAY_0,
        AluInp.PREV_DELAY_1,
        AluInp.PREV_DELAY_2,
        AluInp.PREV_DELAY_3,
        AluInp.PREV_DELAY_4,
        AluInp.PREV_DELAY_5,
    ]
    COUNT_ONCE = (Trigger.COUNT, Trigger.NONE, Trigger.NONE)
    SRC_DONE = (Trigger.SRC_TENSOR_DONE, Trigger.NONE, Trigger.NONE)

    def wire_inputs(u):
        u.enable_input(InpSel.SRC_0, 0)
        u.enable_input(InpSel.SRC_0_HI, 1)
        if nports >= 2:
            u.enable_input(InpSel.SRC_1, 2)
        if nports == 4:
            u.enable_input(InpSel.SRC_1_HI, 3)
        u.enable_input(InpSel.CONST_2, 4)

    def passthroughs():
        dp = [UopDpConfig() for _ in range(8)]
        for i in range(8):
            dp[i].pass_through_delay(0, 1, 2, 3, 4, 5)
        return dp

    def steady():
        dp = [UopDpConfig() for _ in range(8)]
        if nports == 4:
            dp[0].pass_through_delay(1, 2, 3, 4, 5)
            dp[0].enable_alu(AluOp.MIN, P_AD, D[0])        # m0 = min(A, B)
            dp[1].pass_through_delay(1, 2, 3, 4, 5)
            dp[1].enable_alu(AluOp.MIN, D[1], D[2])        # m1 = min(C, D)
            dp[1].enable_delay_from_src(DelayInp.PREV_ALU_OUT, 0)  # d0 <- m0
            dp[2].pass_through_delay(1, 2, 3, 4, 5)
            dp[2].enable_alu(AluOp.MIN, P_AD, D[0])        # m = min(m1, m0)
            first_bypass = 3
        elif nports == 2:
            dp[0].pass_through_delay(0, 1, 3, 4, 5)
            dp[0].enable_alu(AluOp.MIN, P_AD, D[1])        # min(A, C)
            first_bypass = 1
        else:
            dp[0].pass_through_delay(1, 2, 3, 4, 5)
            dp[0].enable_alu(AluOp.MIN, P_AD, D[0])        # min(A, B)
            first_bypass = 1
        for i in range(first_bypass, 7):
            dp[i].pass_through_delay(0, 1, 2, 3, 4, 5)
            dp[i].enable_alu(AluOp.BYPASS, P_AD, P_AD)
        dp[7].pass_through_delay(0, 1, 2, 3, 4, 5)
        dp[7].enable_alu(AluOp.MIN, CUR, P_AD)             # acc
        return dp

    sd = UopConfig(trigger=COUNT_ONCE, repeat_count=1, next_uop=(1, 0, 0))
    wire_inputs(sd)
    dp = passthroughs()
    dp[7] = UopDpConfig()
    dp[7].pass_through_delay(0, 1, 2, 4, 5)
    dp[7].enable_alu(AluOp.BYPASS, D[3], D[3])             # seed <- BIG
    sd.datapath_config = dp

    st = UopConfig(
        trigger=SRC_DONE,
        require_inp0=1,
        require_inp1=1 if nports >= 2 else 0,
    )
    wire_inputs(st)
    st.datapath_config = steady()
    st.enable_output(OutSel.ALU_OUT, OutPath.WR0_LO)
    st.enable_output(OutSel.ALU_OUT, OutPath.WR0_HI)
    if nports >= 2:
        st.enable_output(OutSel.ALU_OUT, OutPath.WR1_LO)
        st.enable_output(OutSel.ALU_OUT, OutPath.WR1_HI)
    return [sd, st]


def _register_ops():
    """Register the masked-min (2X) and pure-min-scan (4X) ops. Idempotent."""
    from concourse import dve_ops
    from concourse.dve_spec import (
        C0,
        C1,
        C2,
        AluOp,
        Bin,
        Spec,
        Src0,
        Src1,
        lower,
        scan,
        select,
    )
    from concourse.dve_uop import DveOpSpec

    def _make(name, spec, build_variants, perf_max, rd1_en):
        for op in dve_ops.OPS:
            if op.name == name:
                return op

        class _Op(dve_ops.DveOp):
            def compile(self, ver):
                key = (self.name, ver)
                if (r := dve_ops._COMPILE_CACHE.get(key)) is not None:
                    return r
                uops = lower(self.spec, ver=ver)
                variants = {}
                if ver == "v3":
                    variants = build_variants()
                    for v in variants.values():
                        assert len(v) == len(uops), (name, len(v), len(uops))
                result = DveOpSpec(
                    name=self.name,
                    opcode=dve_ops.get_dve_sub_opcode(self.name),
                    uops=uops,
                    uops_2x=variants.get("2x"),
                    uops_2x_2p=variants.get("2x_2p"),
                    uops_4x=variants.get("4x"),
                    perf_max=perf_max if variants else 0,
                    rd1_en=rd1_en,
                )
                dve_ops._COMPILE_CACHE[key] = result
                return result

        shas: dict = {}
        op = _Op(name, spec, subdim=False, uops_sha=shas)
        dve_ops.OPS.append(op)
        row = dve_ops._CUSTOM_DVE_ROW_BASE + len(dve_ops.OPS) - 1
        dve_ops._SUB_OPCODE_FOR_NAME[name] = row
        dve_ops.CUSTOM_DVE_SPECS[name] = spec
        for ver in ("v3", "v4"):
            shas[ver] = op.compile(ver).sha(ver) if ver == "v3" else ""
        return op

    def _mm_ref(in0, in1, s0, s1, imm2):
        mask = np.abs(in1.astype(np.float32) - np.asarray(s0, np.float32)) <= (
            np.float32(s1)
        )
        o = np.where(mask, in0.astype(np.float32), np.float32(imm2)).astype(
            np.float32
        )
        return np.minimum.accumulate(o, axis=-1)

    mm_spec = Spec(
        body=scan(
            AluOp.MIN,
            select(C1 >= Bin(AluOp.ABSOLUTE_DIFF, Src1, C0), Src0, C2),
            init=C2,
        ),
        reference=_mm_ref,
    )
    mm = _make(
        "CLUSTER_MASKMIN_ANT",
        mm_spec,
        lambda: {"2x": _build_maskmin_2x_uops()},
        perf_max=1,
        rd1_en=True,
    )

    def _sc_ref(in0, in1, s0, s1, imm2):
        o = np.minimum.accumulate(in0.astype(np.float32), axis=-1)
        return np.minimum(o, np.float32(imm2))

    sc_spec = Spec(body=scan(AluOp.MIN, Src0, init=C2), reference=_sc_ref)
    sc = _make(
        "PUREMIN_SCAN_ANT",
        sc_spec,
        lambda: {
            "2x": _build_scan_uops(1),
            "2x_2p": _build_scan_uops(2),
            "4x": _build_scan_uops(4),
        },
        perf_max=3,
        rd1_en=False,
    )
    return mm, sc


# --------------------------------------------------------------- host plan
def _exact_bounds(psort, lab):
    """Exact contiguous mask range per row: first/last sorted-proto index j
    with |f32(psort[j] - lab)| <= 0.5 (f32 predicate identical to the
    reference).  searchsorted gives a 1-ulp-accurate seed; refine locally."""
    lab = lab.astype(np.float32)
    n = len(psort)
    lo = np.searchsorted(psort, (lab - np.float32(0.5)).astype(np.float32),
                         side="left").astype(np.int64)
    hi = np.searchsorted(psort, (lab + np.float32(0.5)).astype(np.float32),
                         side="right").astype(np.int64)

    def pred(idx):
        idxc = np.clip(idx, 0, n - 1)
        v = np.abs((psort[idxc] - lab).astype(np.float32)) <= np.float32(0.5)
        return v & (idx >= 0) & (idx < n)

    for _ in range(3):
        lo = np.where(pred(lo - 1), lo - 1, lo)       # extend left
    for _ in range(3):
        shrink = ~pred(lo) & (lo < hi)
        lo = np.where(shrink, lo + 1, lo)             # shrink left
    for _ in range(3):
        hi = np.where(pred(hi), hi + 1, hi)           # extend right
    for _ in range(3):
        shrink = ~pred(hi - 1) & (hi > lo)
        hi = np.where(shrink, hi - 1, hi)             # shrink right
    hi = np.maximum(hi, lo)
    return lo, hi


def _plan(labels, proto):
    labels = np.asarray(labels, np.float32)
    proto = np.asarray(proto, np.float32)
    colperm = np.argsort(proto, kind="stable")
    psort = proto[colperm]
    roworder = np.argsort(labels, kind="stable")
    lo, hi = _exact_bounds(psort, labels[roworder])

    glo = lo.reshape(NG, 128)
    ghi = hi.reshape(NG, 128)
    gBLO = (glo.min(axis=1) // ALIGN) * ALIGN
    gBHI = -(-ghi.max(axis=1) // ALIGN) * ALIGN
    gW = np.maximum(gBHI - gBLO, ALIGN)

    rank = np.argsort(-gW, kind="stable")
    W_t = np.zeros(RT, np.int64)
    assign = np.zeros((NCORES, RT), np.int64)
    for t in range(RT):
        grp = rank[NCORES * t:NCORES * (t + 1)]
        W_t[t] = gW[grp].max()
        assign[:, t] = grp

    ILO_t = np.zeros(RT, np.int64)
    IHI_t = np.zeros(RT, np.int64)
    a_loc = np.zeros((NCORES, RT, 128), np.int64)
    b_loc = np.zeros((NCORES, RT, 128), np.int64)
    for t in range(RT):
        ilo, ihi = 0, 1 << 40
        for c in range(NCORES):
            g = assign[c, t]
            a = glo[g] - gBLO[g]
            b = ghi[g] - gBLO[g]
            a_loc[c, t] = a
            b_loc[c, t] = b
            ilo = max(ilo, -(-a.max() // ALIGN) * ALIGN)
            ihi = min(ihi, (b.min() // ALIGN) * ALIGN)
        ihi = min(ihi, int(W_t[t]))
        ilo = min(ilo, int(W_t[t]))
        if ihi < ilo:
            ihi = ilo
        ILO_t[t], IHI_t[t] = ilo, ihi

    # per-tile DMA split point (multiple of ALIGN, inside [ILO, IHI])
    MID_t = np.zeros(RT, np.int64)
    for t in range(RT):
        m = (int(W_t[t]) // 2 // ALIGN) * ALIGN
        MID_t[t] = min(max(m, int(ILO_t[t])), int(IHI_t[t]))
    # op list (half-gated order): (tile, kind, width, half)
    # kind 0=interior lo half, 3=interior hi half, 1=left band, 2=right band
    ops = []
    for t in range(RT):
        if MID_t[t] > ILO_t[t]:
            ops.append((t, 0, int(MID_t[t] - ILO_t[t]), 0))
        if IHI_t[t] > MID_t[t]:
            ops.append((t, 3, int(IHI_t[t] - MID_t[t]), 1))
        if ILO_t[t] > 0:
            ops.append((t, 1, int(ILO_t[t]), 0))
        if W_t[t] > IHI_t[t]:
            ops.append((t, 2, int(W_t[t] - IHI_t[t]), 1))
    rampw = max(
        [8] + [w for (_, k, w, _h) in ops if k in (1, 2)]
    )
    rampw = -(-rampw // ALIGN) * ALIGN
    return dict(colperm=colperm, roworder=roworder, gBLO=gBLO,
                W_t=W_t, assign=assign, ILO_t=ILO_t, IHI_t=IHI_t,
                MID_t=MID_t, a_loc=a_loc, b_loc=b_loc, ops=ops, rampw=rampw)


# --------------------------------------------------------------- device
def _get_bass(pl):
    key = ("v21", tuple(pl["W_t"]), tuple(pl["ILO_t"]), tuple(pl["IHI_t"]),
           tuple(pl["MID_t"]), pl["rampw"])
    if key in _cache:
        return _cache[key]
    _ensure_path()
    import concourse.bacc as bacc
    import concourse.mybir as mybir

    mm_op, sc_op = _register_ops()
    bf16 = mybir.dt.bfloat16
    f16 = mybir.dt.float16

    W_t = [int(x) for x in pl["W_t"]]
    ILO = [int(x) for x in pl["ILO_t"]]
    IHI = [int(x) for x in pl["IHI_t"]]
    MID = [int(x) for x in pl["MID_t"]]
    ops = pl["ops"]
    NOPS = len(ops)
    SUMW = sum(W_t)
    RAMPW = int(pl["rampw"])
    AUXW = RAMPW + 4 * RT  # trailing 2*RT f32 consts stored as f16 pairs
    MAXW = max(W_t)
    SCRW = MAXW + 4 * NOPS + 8

    nc = bacc.Bacc(
        "TRN2", target_bir_lowering=False, debug=False, num_devices=NCORES
    )
    d_ap = nc.dram_tensor("d", [128, SUMW], bf16, kind="ExternalInput").ap()
    aux_ap = nc.dram_tensor("aux", [128, AUXW], f16, kind="ExternalInput").ap()
    out_ap = nc.dram_tensor("rmin", [128, 4 * NOPS], bf16,
                            kind="ExternalOutput").ap()

    dbig = nc.alloc_sbuf_tensor("dbig_t", [128, SUMW], bf16).ap()
    scr = nc.alloc_sbuf_tensor("scr_t", [128, SCRW], bf16).ap()
    aux = nc.alloc_sbuf_tensor("aux_t", [128, AUXW], f16).ap()
    ramp = aux[:, :RAMPW]
    aux32 = aux.bitcast(mybir.dt.float32)  # [128, AUXW // 2]
    consL = aux32[:, RAMPW // 2:RAMPW // 2 + RT]
    consR = aux32[:, RAMPW // 2 + RT:RAMPW // 2 + 2 * RT]

    d_sems = [nc.alloc_semaphore(f"d{t}h{h}") for t in range(RT)
              for h in range(2)]
    aux_sem = nc.alloc_semaphore("aux")
    dve_sem = nc.alloc_semaphore("dve")
    out_sem = nc.alloc_semaphore("out")

    off_t = np.concatenate([[0], np.cumsum(W_t)])
    # DMA plan: (tile, half) halves split at MID; aux inserted after tile 1
    dma_list = []
    for t in range(RT):
        o = int(off_t[t])
        dma_list.append((t, 0, o, o + MID[t]))
        dma_list.append((t, 1, o + MID[t], o + W_t[t]))

    with nc.Block() as block:

        @block.sync
        def _(sync):
            sync.dma_start(aux[:], aux_ap[:]).then_inc(aux_sem, 16)
            for t, h, lo, hi in dma_list:
                if hi > lo:
                    # every op's gating half is nonempty by construction;
                    # an empty half has no waiter, so just skip its DMA
                    sync.dma_start(
                        dbig[:, lo:hi], d_ap[:, lo:hi]
                    ).then_inc(d_sems[2 * t + h], 16)
            sync.wait_ge(dve_sem, NOPS)
            sync.dma_start(
                out_ap[:], scr[:, SCRW - 4 * NOPS:SCRW]
            ).then_inc(out_sem, 16)
            sync.wait_ge(out_sem, 16)
            all_sems = sorted(
                s.num for s in [*d_sems, aux_sem, dve_sem, out_sem]
            )
            lo = prev = all_sems[0]
            for n in all_sems[1:] + [None]:
                if n is not None and n == prev + 1:
                    prev = n
                    continue
                sync.sem_clear(range(lo, prev + 1))
                if n is not None:
                    lo = prev = n

        @block.vector
        def _(vector):
            waited_aux = False
            waited = set()
            for k, (t, kind, w, h) in enumerate(ops):
                if (t, h) not in waited:
                    vector.wait_ge(d_sems[2 * t + h], 16)
                    waited.add((t, h))
                if kind in (1, 2) and not waited_aux:
                    vector.wait_ge(aux_sem, 16)
                    waited_aux = True
                end = SCRW - 4 * k
                o = int(off_t[t])
                if kind == 0:
                    inst = nc.vector._custom_dve(
                        sc_op,
                        out=scr[:, end - w:end],
                        in0=dbig[:, o + ILO[t]:o + MID[t]],
                        imm2=float(BIG),
                    )
                    inst.ins.perf_max = 3
                elif kind == 3:
                    inst = nc.vector._custom_dve(
                        sc_op,
                        out=scr[:, end - w:end],
                        in0=dbig[:, o + MID[t]:o + IHI[t]],
                        imm2=float(BIG),
                    )
                    inst.ins.perf_max = 3
                elif kind == 1:
                    inst = nc.vector._custom_dve(
                        mm_op,
                        out=scr[:, end - w:end],
                        in0=dbig[:, o:o + w],
                        in1=ramp[:, :w],
                        s0=consL[:, t:t + 1],
                        s1=float(HALF_W),
                        imm2=float(BIG),
                    )
                    inst.ins.perf_max = 1
                else:
                    inst = nc.vector._custom_dve(
                        mm_op,
                        out=scr[:, end - w:end],
                        in0=dbig[:, o + IHI[t]:o + W_t[t]],
                        in1=ramp[:, :w],
                        s0=consR[:, t:t + 1],
                        s1=float(HALF_W),
                        imm2=float(BIG),
                    )
                    inst.ins.perf_max = 1
                inst.then_inc(dve_sem, 1)

    nc.compile()
    _cache[key] = nc
    return nc


# --------------------------------------------------------------- staging
def _stage(min_distances, labels, proto_classes, pl):
    import ml_dtypes

    bf16 = ml_dtypes.bfloat16
    d = np.asarray(min_distances, np.float32)
    dcols = np.ascontiguousarray(d[:, pl["colperm"]])
    W_t = pl["W_t"]
    rampw = int(pl["rampw"])
    ramp = np.arange(rampw, dtype=np.float16)

    in_maps = []
    for c in range(NCORES):
        segs = []
        aux = np.zeros((128, rampw + 4 * RT), np.float16)
        aux[:, :rampw] = ramp[None, :]
        consf32 = aux[:, rampw:].view(np.float32)  # [128, 2*RT]
        for t in range(RT):
            g = int(pl["assign"][c, t])
            rows = pl["roworder"][128 * g:128 * (g + 1)]
            blo = int(pl["gBLO"][g])
            w = int(W_t[t])
            seg = np.full((128, w), BIG, np.float32)
            real = max(0, min(w, P - blo))
            seg[:, :real] = dcols[rows, blo:blo + real]
            segs.append(seg)
            a = pl["a_loc"][c, t].astype(np.float32)
            bb = (pl["b_loc"][c, t] - pl["IHI_t"][t]).astype(np.float32)
            consf32[:, t] = a + np.float32(HALF_W)
            consf32[:, RT + t] = bb - np.float32(1.0) - np.float32(HALF_W)
        dcat = np.concatenate(segs, axis=1).astype(bf16)
        in_maps.append(
            {"d": np.ascontiguousarray(dcat), "aux": np.ascontiguousarray(aux)}
        )
    return in_maps


def kernel(min_distances, labels, proto_classes):
    _ensure_path()
    pl = _plan(labels, proto_classes)
    nc = _get_bass(pl)
    from concourse.bass_utils import run_bass_kernel_spmd

    in_maps = _stage(min_distances, labels, proto_classes, pl)
    res = run_bass_kernel_spmd(
        nc, in_maps, core_ids=list(range(NCORES))
    ).results

    ops = pl["ops"]
    NOPS = len(ops)
    loss_rows = np.zeros(B, np.float64)
    acc = np.full((NCORES, RT, 128), np.float32(BIG), np.float32)
    for c in range(NCORES):
        r = np.asarray(res[c]["rmin"]).astype(np.float32)  # [128, 4*NOPS]
        for k, (t, kind, w, h) in enumerate(ops):
            # op k's final value: scratch col (SCRW - 4k - 1) -> gather-local
            col = 4 * (NOPS - k) - 1
            acc[c, t] = np.minimum(acc[c, t], r[:, col])
    for c in range(NCORES):
        for t in range(RT):
            g = int(pl["assign"][c, t])
            rows = pl["roworder"][128 * g:128 * (g + 1)]
            dmin = acc[c, t]
            lr = np.where(
                dmin >= np.float32(BIG / 2),
                np.float32(128.0),
                (MAX_DIST - (MAX_DIST - dmin).astype(np.float32)).astype(
                    np.float32
                ),
            )
            loss_rows[rows] = lr
    return np.array(loss_rows.mean(dtype=np.float64), dtype=np.float32)
